# revision 1
# baseline (speedup 1.0000x reference)
"""DirectedDualPNA on 8 Trainium2 NeuronCores.

Strategy (node-sharded):
  m_e = h_e @ pre_W + pre_b with h_e = [x_dst | x_src] decomposes as
  m_e = A[dst] + B[src],  A = x @ pre_W[:F] + pre_b,  B = x @ pre_W[F:].
  Per-dst segment stats of m then reduce to segment stats of B[src]:
    sum   = cnt*A + sum(B);   mean = sum/safe
    var   = E[B^2] - E[B]^2   (A shifts cancel)
    min   = A + min(B); max = A + max(B)   (0 when cnt==0)
  So only B rows (512B) are gathered per edge (dma_gather, int16 idx via a
  lo/hi table split), aggregated per 128-node groups (nodes degree-sorted so
  group slot-counts are tight), then the node-level PNA tail (post/lin via
  PE matmuls with the three degree scalers applied to matmul outputs).
  Each core owns N/8 dst nodes; B tables are computed replicated on every
  core; one AllGather moves layer-1 output between layers.
"""

import os
import numpy as np

import concourse.bass as bass
import concourse.mybir as mybir
from concourse import bacc
from concourse.bass_utils import run_bass_kernel_spmd
from concourse.tile import TileContext
from concourse.masks import make_identity

P = 128
F = 128
NCORES = 8
LAYERS = 2
EPS = 1e-5
BIG = 1e30
FP32 = mybir.dt.float32
I16 = mybir.dt.int16


# ----------------------------------------------------------------- host prep

def _wrap16(flat):
    """[n] int16 -> wrapped [128, n//16]: position j lives at (j%16, j//16),
    replicated across the 8 Q7 cores (every 16 partitions)."""
    n = flat.shape[0]
    assert n % 16 == 0
    w = flat.reshape(n // 16, 16).T.astype(np.int16)
    return np.tile(w, (8, 1))


def _prep_direction(edge_index, n_nodes, nown, half, avg_log):
    """Host-side per-direction prep. Returns per-core dict + shared K sched."""
    src = np.asarray(edge_index[0], dtype=np.int64)
    dst = np.asarray(edge_index[1], dtype=np.int64)
    nownp = ((nown + P - 1) // P) * P
    ng = nownp // P
    cores = []
    for c in range(NCORES):
        sel = (dst >= c * nown) & (dst < (c + 1) * nown)
        es = src[sel]
        ed = dst[sel] - c * nown
        cnt = np.bincount(ed, minlength=nown)
        order = np.argsort(ed, kind="stable")
        es_sorted = es[order]
        starts = np.zeros(nown + 1, np.int64)
        np.cumsum(cnt, out=starts[1:])
        perm = np.argsort(-cnt, kind="stable")
        lo_lists = []
        hi_lists = []
        for j in range(nownp):
            if j < nown:
                n = perm[j]
                s = es_sorted[starts[n]:starts[n + 1]]
                lo_lists.append(s[s < half])
                hi_lists.append(s[s >= half])
            else:
                lo_lists.append(np.empty(0, np.int64))
                hi_lists.append(np.empty(0, np.int64))
        clo = np.array([len(v) for v in lo_lists], np.int64)
        chi = np.array([len(v) for v in hi_lists], np.int64)
        cores.append(dict(cnt=cnt, perm=perm, lo=lo_lists, hi=hi_lists,
                          clo=clo, chi=chi))
    # shared slot schedule
    K_lo = np.zeros(ng, np.int64)
    K_hi = np.zeros(ng, np.int64)
    for g in range(ng):
        s, e = g * P, (g + 1) * P
        K_lo[g] = max(1, max(int(cc["clo"][s:e].max()) for cc in cores))
        K_hi[g] = max(1, max(int(cc["chi"][s:e].max()) for cc in cores))
    dummy_hi = n_nodes - half  # row `n_nodes` of the table, in hi-half coords
    out_cores = []
    for c in range(NCORES):
        cc = cores[c]
        idx_lo_parts = []
        idx_hi_parts = []
        scal = np.zeros((ng, P, 16), np.float32)
        for g in range(ng):
            kl, kh = int(K_lo[g]), int(K_hi[g])
            slo = np.zeros((kl, P), np.int64)
            shi = np.full((kh, P), dummy_hi, np.int64)
            for p in range(P):
                j = g * P + p
                lo, hi = cc["lo"][j], cc["hi"][j]
                nl_, nh_ = len(lo), len(hi)
                if nl_ > 0:
                    slo[:nl_, p] = lo
                    slo[nl_:, p] = lo[0]
                # else stays 0 (row 0; masked + sum-corrected)
                if nh_ > 0:
                    shi[:nh_, p] = hi - half
                    shi[nh_:, p] = hi[0] - half
                cntj = cc["cnt"][cc["perm"][j]] if j < nown else 0
                safe = max(cntj, 1)
                logd = np.log(safe + 1.0)
                scal[g, p, 0] = -(kl - nl_)          # npadlo
                scal[g, p, 1] = -(kh - nh_)          # npadhi
                scal[g, p, 2] = 0.0 if nl_ > 0 else BIG   # mlo_big
                scal[g, p, 3] = 0.0 if nh_ > 0 else BIG   # mhi_big
                scal[g, p, 4] = 1.0 / safe           # recip
                scal[g, p, 5] = 1.0 if cntj > 0 else 0.0  # nonempty
                scal[g, p, 6] = logd / avg_log       # f1
                scal[g, p, 7] = avg_log / logd       # f2
                scal[g, p, 8] = float(cntj)          # cnt
            idx_lo_parts.append(_wrap16(slo.reshape(-1).astype(np.int16)))
            idx_hi_parts.append(_wrap16(shi.reshape(-1).astype(np.int16)))
        perm_pad = np.concatenate([cc["perm"], np.zeros(nownp - nown, np.int64)])
        # AX gather: pos c*128+p -> row perm_pad[c*128+p] of natural A table
        ax_idx = _wrap16(perm_pad.astype(np.int16))
        # h scatter: pos c*128+p (perm position) -> natural row perm_pad[...]
        hdest = perm_pad.copy()
        hdest[nown:] = nownp  # dummy row
        h_idx = _wrap16(hdest.astype(np.int16))
        out_cores.append(dict(
            idx_lo=np.concatenate(idx_lo_parts, axis=1),
            idx_hi=np.concatenate(idx_hi_parts, axis=1),
            scal=scal, ax_idx=ax_idx, h_idx=h_idx,
        ))
    return out_cores, K_lo, K_hi, ng, nownp


def _prep_weights(inputs, l):
    """Per-layer/direction packed weights (numpy)."""
    w = {}
    for d, tag in enumerate(("in", "out")):
        preW = np.asarray(inputs[f"{tag}_pre_W"][l], np.float32)    # [256,128]
        preB = np.asarray(inputs[f"{tag}_pre_b"][l], np.float32)    # [128]
        postW = np.asarray(inputs[f"{tag}_post_W"][l], np.float32)  # [2048,64]
        postB = np.asarray(inputs[f"{tag}_post_b"][l], np.float32)  # [64]
        linW = np.asarray(inputs[f"{tag}_lin_W"][l], np.float32)    # [64,64]
        linB = np.asarray(inputs[f"{tag}_lin_b"][l], np.float32)    # [64]
        w[(d, "wbot")] = preW[F:2 * F]                               # [128,128]
        w[(d, "acatw")] = np.concatenate([preW[0:F], postW[0:F]], axis=1)  # [128,192]
        w[(d, "acatb")] = np.tile(np.concatenate([preB, postB])[None, :], (P, 1))
        pp = np.zeros((F, 5 * 192), np.float32)
        for p_ in range(5):
            for k in range(3):
                rows = postW[F + k * 5 * F + p_ * F: F + k * 5 * F + (p_ + 1) * F]
                pp[:, p_ * 192 + k * 64: p_ * 192 + (k + 1) * 64] = rows
        w[(d, "ppw")] = pp
        w[(d, "linw")] = linW
        w[(d, "linb")] = np.tile(linB[None, :], (P, 1))
    w["combw"] = np.asarray(inputs["comb_W"][l], np.float32)         # [256,128]
    w["combb"] = np.tile(np.asarray(inputs["comb_b"][l], np.float32)[None, :], (P, 1))
    return w


# -------------------------------------------------------------- device build

def _seg_fold(nc, scratch, src_tile, off, K, op, out_ap, vec):
    """out[128,F] = reduce(src_tile[:, off : off+K*F] viewed [K,F], op) along K.
    Pure-DVE contiguous fold tree (odd blocks deferred, no cross-engine
    copies)."""
    tt = vec.tensor_tensor
    if K == 1:
        vec.tensor_copy(out_ap, src_tile[:, off:off + F])
        return
    if K == 2:
        tt(out_ap, src_tile[:, off:off + F], src_tile[:, off + F:off + 2 * F], op=op)
        return
    if K == 3:
        tt(scratch[:, 0:F], src_tile[:, off:off + F],
           src_tile[:, off + F:off + 2 * F], op=op)
        tt(out_ap, scratch[:, 0:F], src_tile[:, off + 2 * F:off + 3 * F], op=op)
        return
    pend = []
    h = K // 2
    tt(scratch[:, 0:h * F], src_tile[:, off:off + h * F],
       src_tile[:, off + h * F:off + 2 * h * F], op=op)
    if K - 2 * h:
        pend.append((src_tile, off + 2 * h * F))
    cur = h
    while cur > 2:
        h2 = cur // 2
        tt(scratch[:, 0:h2 * F], scratch[:, 0:h2 * F],
           scratch[:, h2 * F:2 * h2 * F], op=op)
        if cur - 2 * h2:
            # odd block at 2*h2*F is never touched by deeper levels
            pend.append((scratch, 2 * h2 * F))
        cur = h2
    if cur == 2:
        operands = [(scratch, 0), (scratch, F)] + pend
    else:
        operands = [(scratch, 0)] + pend
    n = len(operands)
    at, ao = operands[0]
    for i in range(1, n):
        bt, bo = operands[i]
        dst = out_ap if i == n - 1 else scratch[:, 0:F]
        tt(dst, at[:, ao:ao + F], bt[:, bo:bo + F], op=op)
        at, ao = scratch, 0
    return


def build_program(meta):
    """Build the SPMD bass program (shared by all 8 cores)."""
    n_nodes = meta["n_nodes"]
    half = meta["half"]
    tbl = meta["tbl"]
    nown = meta["nown"]
    nownp = meta["nownp"]
    ng = meta["ng"]
    K_lo = meta["K_lo"]    # [L? no: per direction] dict d -> [ng]
    K_hi = meta["K_hi"]
    sum_klo = {d: int(K_lo[d].sum()) for d in (0, 1)}
    sum_khi = {d: int(K_hi[d].sum()) for d in (0, 1)}
    maxK = max(max(int(K_lo[d].max()), int(K_hi[d].max())) for d in (0, 1))
    ntile_x = tbl // P              # B-table build tiles
    ntile_x2 = (n_nodes + P - 1) // P   # x2 full tiles (rest zeroed)
    HT = nownp + P                  # h table rows (incl dummy)

    nc = bacc.Bacc("TRN2", target_bir_lowering=False, debug=False,
                   num_devices=NCORES)

    # ---- DRAM I/O
    xT = nc.dram_tensor("xT", [P, tbl], FP32, kind="ExternalInput")
    xTown = nc.dram_tensor("xTown", [P, nownp], FP32, kind="ExternalInput")
    idx_lo = {d: nc.dram_tensor(f"idx_lo{d}", [P, sum_klo[d] * 8], I16, kind="ExternalInput") for d in (0, 1)}
    idx_hi = {d: nc.dram_tensor(f"idx_hi{d}", [P, sum_khi[d] * 8], I16, kind="ExternalInput") for d in (0, 1)}
    scal_t = {d: nc.dram_tensor(f"scal{d}", [ng, P, 16], FP32, kind="ExternalInput") for d in (0, 1)}
    ax_idx_t = {d: nc.dram_tensor(f"axidx{d}", [P, nownp // 16], I16, kind="ExternalInput") for d in (0, 1)}
    h_idx_t = {d: nc.dram_tensor(f"hidx{d}", [P, nownp // 16], I16, kind="ExternalInput") for d in (0, 1)}
    wbot_t = nc.dram_tensor("wbot", [LAYERS, 2, F, F], FP32, kind="ExternalInput")
    acatw_t = nc.dram_tensor("acatw", [LAYERS, 2, F, 192], FP32, kind="ExternalInput")
    acatb_t = nc.dram_tensor("acatb", [LAYERS, 2, P, 192], FP32, kind="ExternalInput")
    ppw_t = nc.dram_tensor("ppw", [LAYERS, 2, F, 5 * 192], FP32, kind="ExternalInput")
    linw_t = nc.dram_tensor("linw", [LAYERS, 2, 64, 64], FP32, kind="ExternalInput")
    linb_t = nc.dram_tensor("linb", [LAYERS, 2, P, 64], FP32, kind="ExternalInput")
    combw_t = nc.dram_tensor("combw", [LAYERS, 256, F], FP32, kind="ExternalInput")
    combb_t = nc.dram_tensor("combb", [LAYERS, P, F], FP32, kind="ExternalInput")
    headw_t = nc.dram_tensor("headw", [F, 8], FP32, kind="ExternalInput")
    headb_t = nc.dram_tensor("headb", [P, 8], FP32, kind="ExternalInput")
    out_t = nc.dram_tensor("out", [nown, 8], FP32, kind="ExternalOutput")

    # ---- DRAM internal
    Bt = {d: nc.dram_tensor(f"Bt{d}", [tbl, F], FP32) for d in (0, 1)}
    Atab = nc.dram_tensor("Atab", [nownp, 192], FP32)
    h_tab = {d: nc.dram_tensor(f"htab{d}", [HT, 64], FP32) for d in (0, 1)}
    x2own_int = nc.dram_tensor("x2own", [nown, F], FP32)
    x2full = nc.dram_tensor("x2full", [n_nodes, F], FP32, addr_space="Shared")

    AF = mybir.ActivationFunctionType
    OP = mybir.AluOpType
    AX_ = mybir.AxisListType

    with TileContext(nc) as tc:
        with tc.tile_pool(name="const", bufs=1) as constp, \
             tc.tile_pool(name="pers", bufs=1) as persp, \
             tc.tile_pool(name="wts", bufs=1) as wtsp, \
             tc.tile_pool(name="xt", bufs=2) as xtp, \
             tc.tile_pool(name="gath", bufs=2) as gathp, \
             tc.tile_pool(name="sqp", bufs=1) as sqp, \
             tc.tile_pool(name="fold", bufs=2) as foldp, \
             tc.tile_pool(name="ip", bufs=3) as ipool, \
             tc.tile_pool(name="nl", bufs=2) as nlp, \
             tc.tile_pool(name="ps", bufs=2, space="PSUM") as psp, \
             tc.tile_pool(name="psa", bufs=2, space="PSUM") as psap:

            ident = constp.tile([P, P], FP32)
            make_identity(nc, ident[:])
            zeros = constp.tile([P, 1024], FP32)
            nc.vector.memset(zeros[:], 0.0)
            eps_col = constp.tile([P, 1], FP32)
            nc.vector.memset(eps_col[:], EPS)

            AXt = persp.tile([P, ng * 192], FP32)         # gathered A|X0 (perm)
            hsb = persp.tile([P, ng * 64], FP32)          # h accumulation (perm)

            def load_w(pool, dram_ap, shape, tag):
                t = pool.tile(shape, FP32, tag=tag)
                nc.sync.dma_start(out=t[:], in_=dram_ap)
                return t

            headw_sb = load_w(constp, headw_t[:], [F, 8], "headw")
            headb_sb = load_w(constp, headb_t[:], [P, 8], "headb")

            def zero_dram(tensor, rows, width):
                flat = tensor[:].rearrange("n f -> (n f)")
                total = rows * width
                assert total % P == 0
                per = total // P
                v = flat.rearrange("(p x) -> p x", p=P)
                off = 0
                while off < per:
                    w = min(1024, per - off)
                    nc.sync.dma_start(out=v[:, off:off + w], in_=zeros[:, 0:w])
                    off += w

            def transpose_to_sbuf(src_ap, rows, cols, tag):
                """PE transpose src [rows, cols] -> sbuf [cols, rows]."""
                pt = psp.tile([P, P], FP32, tag="ptr")
                nc.tensor.transpose(out=pt[:cols, :rows], in_=src_ap, identity=ident[:])
                sb = xtp.tile([P, P], FP32, tag=tag)
                nc.vector.tensor_copy(sb[:cols, :rows], pt[:cols, :rows])
                return sb

            for l in range(LAYERS):
                wbot_sb = {d: load_w(wtsp, wbot_t[l, d], [F, F], f"wbot{d}") for d in (0, 1)}
                combw1_sb = load_w(wtsp, combw_t[l, 0:F, :], [F, F], "combw1")
                combw2_sb = load_w(wtsp, combw_t[l, F:256, :], [F, F], "combw2")
                combb_sb = load_w(wtsp, combb_t[l], [P, F], "combb")

                # ---------- phase A: B tables (both directions)
                for t in range(ntile_x if l == 0 else ntile_x2):
                    if l == 0:
                        lhsT = xtp.tile([P, P], FP32, tag="xtt")
                        nc.sync.dma_start(out=lhsT[:], in_=xT[:, t * P:(t + 1) * P])
                    else:
                        rows = min(P, n_nodes - t * P)
                        x2t = xtp.tile([P, P], FP32, tag="x2t")
                        if rows < P:
                            nc.vector.memset(x2t[:], 0.0)
                        nc.sync.dma_start(out=x2t[:rows, :],
                                          in_=x2full[t * P:t * P + rows, :])
                        lhsT = transpose_to_sbuf(x2t[:], P, P, "xtt")
                    for d in (0, 1):
                        pb = psp.tile([P, P], FP32, tag="pb")
                        nc.tensor.matmul(out=pb[:], lhsT=lhsT[:], rhs=wbot_sb[d][:],
                                         start=True, stop=True)
                        sb = xtp.tile([P, P], FP32, tag="bsb")
                        nc.scalar.copy(sb[:], pb[:])
                        rows_o = min(P, tbl - t * P)
                        nc.sync.dma_start(out=Bt[d][t * P:t * P + rows_o, :],
                                          in_=sb[:rows_o, :])
                if l == 1:
                    # zero rows n_nodes..tbl of both tables (dummy row etc.)
                    for d in (0, 1):
                        r = n_nodes
                        while r < tbl:
                            w = min(P, tbl - r)
                            nc.sync.dma_start(
                                out=Bt[d][r:r + w, :],
                                in_=zeros[:w, 0:F])
                            r += w

                for d in (0, 1):
                    # ---------- phase B: A|X0 natural table + perm gather
                    acatw_sb = load_w(wtsp, acatw_t[l, d], [F, 192], "acatw")
                    acatb_sb = load_w(wtsp, acatb_t[l, d], [P, 192], "acatb")
                    ppw_sb = load_w(wtsp, ppw_t[l, d], [F, 5 * 192], "ppw")
                    linw_sb = load_w(wtsp, linw_t[l, d], [64, 64], "linw")
                    linb_sb = load_w(wtsp, linb_t[l, d], [P, 64], "linb")

                    for g in range(ng):
                        if l == 0:
                            lhsT = xtp.tile([P, P], FP32, tag="xtt")
                            nc.sync.dma_start(out=lhsT[:],
                                              in_=xTown[:, g * P:(g + 1) * P])
                        else:
                            rows_b = min(P, nown - g * P)
                            x2t = xtp.tile([P, P], FP32, tag="x2t")
                            if rows_b < P:
                                nc.vector.memset(x2t[:], 0.0)
                            nc.sync.dma_start(out=x2t[:rows_b, :],
                                              in_=x2own_int[g * P:g * P + rows_b, :])
                            lhsT = transpose_to_sbuf(x2t[:], P, P, "xtt")
                        pa = psap.tile([P, 192], FP32, tag="pa")
                        nc.tensor.matmul(out=pa[:], lhsT=lhsT[:], rhs=acatw_sb[:],
                                         start=True, stop=True)
                        arow = xtp.tile([P, 192], FP32, tag="arow")
                        nc.vector.tensor_add(arow[:], pa[:], acatb_sb[:])
                        nc.sync.dma_start(out=Atab[g * P:(g + 1) * P, :], in_=arow[:])
                    axi = ipool.tile([P, nownp // 16], I16, tag="axi")
                    nc.sync.dma_start(out=axi[:], in_=ax_idx_t[d][:])
                    nc.gpsimd.dma_gather(
                        out_ap=AXt[:].rearrange("p (c w) -> p c w", w=192),
                        in_ap=Atab[:], idxs_ap=axi[:],
                        num_idxs=nownp, num_idxs_reg=nownp,
                        elem_size=192, single_packet=False)

                    # ---------- phase C: gather + stats + post/lin per group
                    off_lo = 0
                    off_hi = 0
                    kl_arr, kh_arr = K_lo[d], K_hi[d]
                    for g in range(ng):
                        KL, KH = int(kl_arr[g]), int(kh_arr[g])
                        Wd = (KL + KH) * F
                        il = ipool.tile([P, KL * 8], I16, tag="il")
                        nc.sync.dma_start(out=il[:], in_=idx_lo[d][:, off_lo:off_lo + KL * 8])
                        ih = ipool.tile([P, KH * 8], I16, tag="ih")
                        nc.sync.dma_start(out=ih[:], in_=idx_hi[d][:, off_hi:off_hi + KH * 8])
                        off_lo += KL * 8
                        off_hi += KH * 8
                        sc = ipool.tile([P, 16], FP32, tag="sc")
                        nc.sync.dma_start(out=sc[:], in_=scal_t[d][g])
                        gt = gathp.tile([P, (maxK * 2) * F], FP32, tag="gt")
                        nc.gpsimd.dma_gather(
                            out_ap=gt[:, 0:KL * F].rearrange("p (k f) -> p k f", f=F),
                            in_ap=Bt[d][0:half, :], idxs_ap=il[:],
                            num_idxs=KL * P, num_idxs_reg=KL * P,
                            elem_size=F, single_packet=False)
                        nc.gpsimd.dma_gather(
                            out_ap=gt[:, KL * F:Wd].rearrange("p (k f) -> p k f", f=F),
                            in_ap=Bt[d][half:tbl, :], idxs_ap=ih[:],
                            num_idxs=KH * P, num_idxs_reg=KH * P,
                            elem_size=F, single_packet=False)
                        fsc = foldp.tile([P, (maxK // 2 + 2) * F], FP32, tag="fsc")

                        def nlt(tag):
                            return nlp.tile([P, F], FP32, tag=tag, name=tag)

                        slo, shi = nlt("slo"), nlt("shi")
                        qlo, qhi = nlt("qlo"), nlt("qhi")
                        mnlo, mnhi = nlt("mnlo"), nlt("mnhi")
                        mxlo, mxhi = nlt("mxlo"), nlt("mxhi")
                        v = nc.vector
                        _seg_fold(nc, fsc, gt, 0, KL, OP.add, slo[:], v)
                        _seg_fold(nc, fsc, gt, KL * F, KH, OP.add, shi[:], v)
                        sq = sqp.tile([P, maxK * F], FP32, tag="sq")
                        nc.scalar.activation(sq[:, 0:KL * F], gt[:, 0:KL * F], AF.Square)
                        _seg_fold(nc, fsc, sq, 0, KL, OP.add, qlo[:], v)
                        sq2 = sqp.tile([P, maxK * F], FP32, tag="sq")
                        nc.scalar.activation(sq2[:, 0:KH * F], gt[:, KL * F:Wd], AF.Square)
                        _seg_fold(nc, fsc, sq2, 0, KH, OP.add, qhi[:], v)
                        _seg_fold(nc, fsc, gt, 0, KL, OP.min, mnlo[:], v)
                        _seg_fold(nc, fsc, gt, KL * F, KH, OP.min, mnhi[:], v)
                        _seg_fold(nc, fsc, gt, 0, KL, OP.max, mxlo[:], v)
                        _seg_fold(nc, fsc, gt, KL * F, KH, OP.max, mxhi[:], v)

                        s0lo = gt[:, 0:F]
                        s0hi = gt[:, KL * F:KL * F + F]
                        q0lo, q0hi = nlt("q0lo"), nlt("q0hi")
                        v.tensor_mul(q0lo[:], s0lo, s0lo)
                        v.tensor_mul(q0hi[:], s0hi, s0hi)
                        npl = sc[:, 0:1]
                        nph = sc[:, 1:2]
                        mlb = sc[:, 2:3]
                        mhb = sc[:, 3:4]
                        rcp = sc[:, 4:5]
                        nemp = sc[:, 5:6]
                        f1 = sc[:, 6:7]
                        f2 = sc[:, 7:8]
                        cntc = sc[:, 8:9]

                        t1, t2 = nlt("t1"), nlt("t2")
                        Sb, SQb = nlt("Sb"), nlt("SQb")
                        v.scalar_tensor_tensor(t1[:], s0lo, npl, slo[:], op0=OP.mult, op1=OP.add)
                        v.scalar_tensor_tensor(t2[:], s0hi, nph, shi[:], op0=OP.mult, op1=OP.add)
                        v.tensor_add(Sb[:], t1[:], t2[:])
                        v.scalar_tensor_tensor(t1[:], q0lo[:], npl, qlo[:], op0=OP.mult, op1=OP.add)
                        v.scalar_tensor_tensor(t2[:], q0hi[:], nph, qhi[:], op0=OP.mult, op1=OP.add)
                        v.tensor_add(SQb[:], t1[:], t2[:])
                        MN, MX = nlt("MN"), nlt("MX")
                        v.tensor_scalar(t1[:], mnlo[:], mlb, None, op0=OP.add)
                        v.tensor_scalar(t2[:], mnhi[:], mhb, None, op0=OP.add)
                        v.tensor_tensor(MN[:], t1[:], t2[:], op=OP.min)
                        v.tensor_scalar(t1[:], mxlo[:], mlb, None, op0=OP.subtract)
                        v.tensor_scalar(t2[:], mxhi[:], mhb, None, op0=OP.subtract)
                        v.tensor_tensor(MX[:], t1[:], t2[:], op=OP.max)

                        Ag = AXt[:, g * 192:g * 192 + F]
                        X0g = AXt[:, g * 192 + F:g * 192 + 192]
                        s_full, mean = nlt("s_full"), nlt("mean")
                        meanB, std = nlt("meanB"), nlt("std")
                        mn, mx = nlt("mn"), nlt("mx")
                        v.scalar_tensor_tensor(s_full[:], Ag, cntc, Sb[:], op0=OP.mult, op1=OP.add)
                        nc.scalar.activation(mean[:], s_full[:], AF.Copy, scale=rcp)
                        nc.scalar.activation(meanB[:], Sb[:], AF.Copy, scale=rcp)
                        nc.scalar.activation(t1[:], SQb[:], AF.Copy, scale=rcp)
                        v.tensor_mul(t2[:], meanB[:], meanB[:])
                        vr1, vr2 = nlt("vr1"), nlt("vr2")
                        v.tensor_sub(vr1[:], t1[:], t2[:])
                        v.tensor_scalar_max(vr2[:], vr1[:], 0.0)
                        nc.scalar.activation(std[:], vr2[:], AF.Sqrt, bias=eps_col[:, 0:1])
                        v.tensor_add(t1[:], Ag, MN[:])
                        v.tensor_scalar(mn[:], t1[:], nemp, None, op0=OP.mult)
                        v.tensor_add(t2[:], Ag, MX[:])
                        v.tensor_scalar(mx[:], t2[:], nemp, None, op0=OP.mult)

                        # post: y = X0 + sum_p sum_k f_k*(part_p @ Wp_k)
                        py = psap.tile([P, 192], FP32, tag="pa")
                        for pi, part in enumerate((mean, s_full, std, mn, mx)):
                            pt = psp.tile([P, P], FP32, tag="ptr")
                            nc.tensor.transpose(out=pt[:], in_=part[:], identity=ident[:])
                            partT = xtp.tile([P, P], FP32, tag="partT")
                            v.tensor_copy(partT[:], pt[:])
                            nc.tensor.matmul(out=py[:], lhsT=partT[:],
                                             rhs=ppw_sb[:, pi * 192:(pi + 1) * 192],
                                             start=(pi == 0), stop=(pi == 4))
                        yt, y64 = nlt("yt"), nlt("y64")
                        pys = nlp.tile([P, 192], FP32, tag="pys", name="pys")
                        nc.scalar.copy(pys[:], py[:])
                        v.scalar_tensor_tensor(yt[:, 0:64], pys[:, 64:128], f1,
                                               pys[:, 0:64], op0=OP.mult, op1=OP.add)
                        v.scalar_tensor_tensor(yt[:, 64:128], pys[:, 128:192], f2,
                                               X0g, op0=OP.mult, op1=OP.add)
                        v.tensor_add(y64[:, 0:64], yt[:, 0:64], yt[:, 64:128])
                        # lin + relu
                        pt = psp.tile([P, P], FP32, tag="ptr")
                        nc.tensor.transpose(out=pt[:64, :], in_=y64[:, 0:64], identity=ident[:])
                        ylhs = xtp.tile([64, P], FP32, tag="ylhs")
                        v.tensor_copy(ylhs[:], pt[:64, :])
                        pz = psp.tile([P, 64], FP32, tag="pz")
                        nc.tensor.matmul(out=pz[:], lhsT=ylhs[:], rhs=linw_sb[:],
                                         start=True, stop=True)
                        zb = nlt("zb")
                        v.tensor_add(zb[:, 0:64], pz[:], linb_sb[:, 0:64])
                        nc.scalar.activation(hsb[:, g * 64:(g + 1) * 64],
                                             zb[:, 0:64], AF.Relu)

                    # scatter h (perm -> natural)
                    zero_dram(h_tab[d], HT, 64)
                    hix = ipool.tile([P, nownp // 16], I16, tag="hix")
                    nc.sync.dma_start(out=hix[:], in_=h_idx_t[d][:])
                    nc.gpsimd.dma_scatter_add(
                        out_ap=h_tab[d][:],
                        in_ap=hsb[:].rearrange("p (c w) -> p c w", w=64),
                        idxs_ap=hix[:], num_idxs=nownp, num_idxs_reg=nownp,
                        elem_size=64, single_packet=False)

                # ---------- phase D: comb (+ head when l==1)
                for g in range(ng):
                    rows = min(P, nown - g * P)
                    if l == 0:
                        xlhs = xtp.tile([P, P], FP32, tag="xtt")
                        nc.sync.dma_start(out=xlhs[:],
                                          in_=xTown[:, g * P:(g + 1) * P])
                    else:
                        x2t = xtp.tile([P, P], FP32, tag="x2t")
                        if rows < P:
                            nc.vector.memset(x2t[:], 0.0)
                        nc.sync.dma_start(out=x2t[:rows, :],
                                          in_=x2own_int[g * P:g * P + rows, :])
                        xlhs = transpose_to_sbuf(x2t[:], P, P, "xtt")
                    hcatT = xtp.tile([P, P], FP32, tag="hcatT")
                    for d in (0, 1):
                        htile = xtp.tile([P, 64], FP32, tag="htile")
                        nc.sync.dma_start(out=htile[:], in_=h_tab[d][g * P:(g + 1) * P, :])
                        pt = psp.tile([P, P], FP32, tag="ptr")
                        nc.tensor.transpose(out=pt[:64, :], in_=htile[:], identity=ident[:])
                        nc.vector.tensor_copy(hcatT[d * 64:(d + 1) * 64, :], pt[:64, :])
                    pc = psp.tile([P, P], FP32, tag="pb")
                    nc.tensor.matmul(out=pc[:], lhsT=xlhs[:], rhs=combw1_sb[:],
                                     start=True, stop=False)
                    nc.tensor.matmul(out=pc[:], lhsT=hcatT[:], rhs=combw2_sb[:],
                                     start=False, stop=True)
                    xn = nlp.tile([P, F], FP32, tag="xn")
                    nc.vector.tensor_add(xn[:], pc[:], combb_sb[:])
                    if l == 0:
                        x2n = nlp.tile([P, F], FP32, tag="x2n", name="x2n")
                        nc.scalar.activation(x2n[:], xn[:], AF.Relu)
                        nc.sync.dma_start(out=x2own_int[g * P:g * P + rows, :],
                                          in_=x2n[:rows, :])
                    else:
                        x3 = nlp.tile([P, F], FP32, tag="x3")
                        nc.scalar.activation(x3[:], xn[:], AF.Relu)
                        pt = psp.tile([P, P], FP32, tag="ptr")
                        nc.tensor.transpose(out=pt[:], in_=x3[:], identity=ident[:])
                        x3T = xtp.tile([P, P], FP32, tag="x3T")
                        nc.vector.tensor_copy(x3T[:], pt[:])
                        ph = psp.tile([P, 8], FP32, tag="pz")
                        nc.tensor.matmul(out=ph[:], lhsT=x3T[:], rhs=headw_sb[:],
                                         start=True, stop=True)
                        ot = nlp.tile([P, 8], FP32, tag="ot")
                        nc.vector.tensor_add(ot[:], ph[:], headb_sb[:])
                        nc.sync.dma_start(out=out_t[g * P:g * P + rows, :],
                                          in_=ot[:rows, :])

                if l == 0:
                    nc.gpsimd.collective_compute(
                        "AllGather", OP.bypass,
                        replica_groups=[list(range(NCORES))],
                        ins=[x2own_int[:]], outs=[x2full[:]])

    nc.finalize()
    return nc


# ----------------------------------------------------------------- kernel()

def _install_ntff_hook():
    """Register the axon NTFF profile hook if the image's antenv lacks it."""
    import sys
    import types
    try:
        from antenv.axon_hooks import get_axon_ntff_profile_hook  # noqa: F401
        return
    except ImportError:
        pass
    try:
        mod = types.ModuleType("antenv.axon_hooks")
        hook = {"h": None}
        mod.set_axon_ntff_profile_hook = lambda h: hook.__setitem__("h", h)
        mod.get_axon_ntff_profile_hook = lambda: hook["h"]
        sys.modules["antenv.axon_hooks"] = mod
        import antenv
        antenv.axon_hooks = mod
        from trn_agent_boot.trn_boot import _ntff_profile_via_ctypes
        mod.set_axon_ntff_profile_hook(
            _ntff_profile_via_ctypes("/opt/axon/libaxon_pjrt.so"))
    except Exception:
        pass


def kernel(**inputs):
    x = np.asarray(inputs["x"], np.float32)
    n_nodes, f = x.shape
    assert f == F
    assert n_nodes % NCORES == 0
    nown = n_nodes // NCORES
    half = ((n_nodes // 2 + 1 + P - 1) // P) * P
    assert half < 32768 and 2 * half > n_nodes
    tbl = 2 * half

    avg_in = float(np.asarray(inputs["avg_in"]))
    avg_out = float(np.asarray(inputs["avg_out"]))

    prep = {}
    Ksched = {}
    for d, (ei, avg) in enumerate(
            ((inputs["edge_index_in"], avg_in), (inputs["edge_index_out"], avg_out))):
        cores, K_lo, K_hi, ng, nownp = _prep_direction(ei, n_nodes, nown, half, avg)
        prep[d] = cores
        Ksched[d] = (K_lo, K_hi)

    meta = dict(n_nodes=n_nodes, half=half, tbl=tbl, nown=nown, nownp=nownp,
                ng=ng, K_lo={d: Ksched[d][0] for d in (0, 1)},
                K_hi={d: Ksched[d][1] for d in (0, 1)})
    nc = build_program(meta)

    xT_np = np.zeros((P, tbl), np.float32)
    xT_np[:, :n_nodes] = x.T
    wl = [_prep_weights(inputs, l) for l in range(LAYERS)]
    wbot_np = np.stack([np.stack([wl[l][(d, "wbot")] for d in (0, 1)]) for l in range(LAYERS)])
    acatw_np = np.stack([np.stack([wl[l][(d, "acatw")] for d in (0, 1)]) for l in range(LAYERS)])
    acatb_np = np.stack([np.stack([wl[l][(d, "acatb")] for d in (0, 1)]) for l in range(LAYERS)])
    ppw_np = np.stack([np.stack([wl[l][(d, "ppw")] for d in (0, 1)]) for l in range(LAYERS)])
    linw_np = np.stack([np.stack([wl[l][(d, "linw")] for d in (0, 1)]) for l in range(LAYERS)])
    linb_np = np.stack([np.stack([wl[l][(d, "linb")] for d in (0, 1)]) for l in range(LAYERS)])
    combw_np = np.stack([wl[l]["combw"] for l in range(LAYERS)])
    combb_np = np.stack([wl[l]["combb"] for l in range(LAYERS)])
    headw_np = np.asarray(inputs["head_W"], np.float32)
    headb_np = np.tile(np.asarray(inputs["head_b"], np.float32)[None, :], (P, 1))

    in_maps = []
    for c in range(NCORES):
        xTown_np = np.zeros((P, meta["nownp"]), np.float32)
        xTown_np[:, :nown] = x[c * nown:(c + 1) * nown].T
        m = dict(xT=xT_np, xTown=xTown_np, wbot=wbot_np, acatw=acatw_np, acatb=acatb_np,
                 ppw=ppw_np, linw=linw_np, linb=linb_np, combw=combw_np,
                 combb=combb_np, headw=headw_np, headb=headb_np)
        for d in (0, 1):
            pc = prep[d][c]
            m[f"idx_lo{d}"] = pc["idx_lo"]
            m[f"idx_hi{d}"] = pc["idx_hi"]
            m[f"scal{d}"] = pc["scal"]
            m[f"axidx{d}"] = pc["ax_idx"]
            m[f"hidx{d}"] = pc["h_idx"]
        in_maps.append(m)

    trace = bool(int(os.environ.get("PNA_TRACE", "0")))
    if trace:
        _install_ntff_hook()
    res = run_bass_kernel_spmd(nc, in_maps, core_ids=list(range(NCORES)),
                               trace=trace)
    if trace and res.exec_time_ns is not None:
        print(f"HW exec time: {res.exec_time_ns} ns")
    out = np.concatenate([res.results[c]["out"] for c in range(NCORES)], axis=0)
    return out.astype(np.float32)


# Optional: expose sim path for debugging (used by test.py on small inputs)
def kernel_sim(**inputs):
    """Single-core-per-core simulation via MultiCoreSim (slow; small inputs)."""
    from concourse.bass_interp import MultiCoreSim
    x = np.asarray(inputs["x"], np.float32)
    n_nodes = x.shape[0]
    nown = n_nodes // NCORES
    half = ((n_nodes // 2 + 1 + P - 1) // P) * P
    tbl = 2 * half
    avg_in = float(np.asarray(inputs["avg_in"]))
    avg_out = float(np.asarray(inputs["avg_out"]))
    prep = {}
    Ksched = {}
    for d, (ei, avg) in enumerate(
            ((inputs["edge_index_in"], avg_in), (inputs["edge_index_out"], avg_out))):
        cores, K_lo, K_hi, ng, nownp = _prep_direction(ei, n_nodes, nown, half, avg)
        prep[d] = cores
        Ksched[d] = (K_lo, K_hi)
    meta = dict(n_nodes=n_nodes, half=half, tbl=tbl, nown=nown, nownp=nownp,
                ng=ng, K_lo={d: Ksched[d][0] for d in (0, 1)},
                K_hi={d: Ksched[d][1] for d in (0, 1)})
    nc = build_program(meta)

    xT_np = np.zeros((P, tbl), np.float32)
    xT_np[:, :n_nodes] = x.T
    wl = [_prep_weights(inputs, l) for l in range(LAYERS)]
    wbot_np = np.stack([np.stack([wl[l][(d, "wbot")] for d in (0, 1)]) for l in range(LAYERS)])
    acatw_np = np.stack([np.stack([wl[l][(d, "acatw")] for d in (0, 1)]) for l in range(LAYERS)])
    acatb_np = np.stack([np.stack([wl[l][(d, "acatb")] for d in (0, 1)]) for l in range(LAYERS)])
    ppw_np = np.stack([np.stack([wl[l][(d, "ppw")] for d in (0, 1)]) for l in range(LAYERS)])
    linw_np = np.stack([np.stack([wl[l][(d, "linw")] for d in (0, 1)]) for l in range(LAYERS)])
    linb_np = np.stack([np.stack([wl[l][(d, "linb")] for d in (0, 1)]) for l in range(LAYERS)])
    combw_np = np.stack([wl[l]["combw"] for l in range(LAYERS)])
    combb_np = np.stack([wl[l]["combb"] for l in range(LAYERS)])
    headw_np = np.asarray(inputs["head_W"], np.float32)
    headb_np = np.tile(np.asarray(inputs["head_b"], np.float32)[None, :], (P, 1))

    sim = MultiCoreSim(nc, num_cores=NCORES, trace=False,
                       require_finite=False, require_nnan=False)
    for c in range(NCORES):
        cs = sim.cores[c]
        cs.tensor("xT")[:] = xT_np
        xTown_np = np.zeros((P, nownp), np.float32)
        xTown_np[:, :nown] = x[c * nown:(c + 1) * nown].T
        cs.tensor("xTown")[:] = xTown_np
        for nm, val in (("wbot", wbot_np), ("acatw", acatw_np), ("acatb", acatb_np),
                        ("ppw", ppw_np), ("linw", linw_np), ("linb", linb_np),
                        ("combw", combw_np), ("combb", combb_np),
                        ("headw", headw_np), ("headb", headb_np)):
            cs.tensor(nm)[:] = val
        for d in (0, 1):
            pc = prep[d][c]
            cs.tensor(f"idx_lo{d}")[:] = pc["idx_lo"]
            cs.tensor(f"idx_hi{d}")[:] = pc["idx_hi"]
            cs.tensor(f"scal{d}")[:] = pc["scal"]
            cs.tensor(f"axidx{d}")[:] = pc["ax_idx"]
            cs.tensor(f"hidx{d}")[:] = pc["h_idx"]
    sim.simulate(check_with_hw=False)
    out = np.concatenate([np.array(sim.cores[c].tensor("out")) for c in range(NCORES)], axis=0)
    return out.astype(np.float32)



# revision 12
# speedup vs baseline: 1.4687x; 1.4687x over previous
"""DirectedDualPNA on 8 Trainium2 NeuronCores — v2.

Strategy (node-sharded):
  m_e = h_e @ pre_W + pre_b with h_e = [x_dst | x_src] decomposes as
  m_e = A[dst] + B[src],  A = x @ pre_W[:F] + pre_b,  B = x @ pre_W[F:].
  Per-dst segment stats of m reduce to segment stats of B[src]:
    sum   = cnt*A + sum(B);   mean = sum/safe
    var   = E[B^2] - E[B]^2   (A shifts cancel)
    min   = A + min(B); max = A + max(B)   (0 when cnt==0)
  Only B rows (512B fp32) are gathered per edge (dma_gather, int16 idx via
  lo/hi table windows), nodes degree-sorted into 128-node groups so slot
  counts are tight.

v2 performance structure:
  - B tables built SHARDED (each core computes its own nodes' rows) then
    AllGather'd — replaces the replicated 392-tile matmul per layer.
  - 4 SWDGE queues, gathers round-robined so 4 DMA paths run concurrently.
  - A|X0 table computed straight into SBUF in perm order (host-permuted
    xT for layer 0; one perm-gather of x2 rows for layer 1) — no AX gather.
  - Segment stats via single-instruction DVE tensor_reduce over strided
    views (sum over lo+hi combined; sumsq/min/max per half).
  - Scalar tail batched per chunk of G=7 groups with stride-0 broadcast
    scalar operands; PSUM->SBUF copies ride the scalar engine.
  - h written back per chunk via dma_scatter_add (perm -> natural).
"""

import os
import numpy as np

import concourse.bass as bass
import concourse.mybir as mybir
from concourse import bacc
from concourse.bass_utils import run_bass_kernel_spmd
from concourse.tile import TileContext
from concourse.masks import make_identity

P = 128
F = 128
NCORES = 8
LAYERS = 2
EPS = 1e-5
BIG = 1e30
G = 4                       # groups per chunk
NQ = 4                      # swdge queues
FP32 = mybir.dt.float32
I16 = mybir.dt.int16


# ----------------------------------------------------------------- host prep

def _wrap16(flat):
    """[n] int16 -> wrapped [128, n//16]: position j lives at (j%16, j//16),
    replicated across the 8 Q7 cores (every 16 partitions)."""
    n = flat.shape[0]
    assert n % 16 == 0
    w = flat.reshape(n // 16, 16).T.astype(np.int16)
    return np.tile(w, (8, 1))


def _chunks_of(ng):
    out = [G] * (ng // G)
    if ng % G:
        out.append(ng % G)
    return out


def _prep_direction(edge_index, n_nodes, nown, half, avg_log):
    """Host-side per-direction prep. Returns per-core dict + shared K sched."""
    src = np.asarray(edge_index[0], dtype=np.int64)
    dst = np.asarray(edge_index[1], dtype=np.int64)
    ng = (nown + P - 1) // P
    nownp = ng * P
    chunks = _chunks_of(ng)
    cores = []
    for c in range(NCORES):
        sel = (dst >= c * nown) & (dst < (c + 1) * nown)
        es = src[sel]
        ed = dst[sel] - c * nown
        cnt = np.bincount(ed, minlength=nown)
        order = np.argsort(ed, kind="stable")
        es_sorted = es[order]
        starts = np.zeros(nown + 1, np.int64)
        np.cumsum(cnt, out=starts[1:])
        perm = np.argsort(-cnt, kind="stable")
        lo_lists = []
        hi_lists = []
        for j in range(nownp):
            if j < nown:
                n = perm[j]
                s = es_sorted[starts[n]:starts[n + 1]]
                lo_lists.append(s[s < half])
                hi_lists.append(s[s >= half])
            else:
                lo_lists.append(np.empty(0, np.int64))
                hi_lists.append(np.empty(0, np.int64))
        clo = np.array([len(v) for v in lo_lists], np.int64)
        chi = np.array([len(v) for v in hi_lists], np.int64)
        cores.append(dict(cnt=cnt, perm=perm, lo=lo_lists, hi=hi_lists,
                          clo=clo, chi=chi))
    # shared slot schedule
    K_lo = np.zeros(ng, np.int64)
    K_hi = np.zeros(ng, np.int64)
    for g in range(ng):
        s, e = g * P, (g + 1) * P
        K_lo[g] = max(1, max(int(cc["clo"][s:e].max()) for cc in cores))
        K_hi[g] = max(1, max(int(cc["chi"][s:e].max()) for cc in cores))
    dummy_hi = n_nodes - half  # row `n_nodes` of the table, in hi-half coords
    nchunks = len(chunks)
    out_cores = []
    for c in range(NCORES):
        cc = cores[c]
        idx_lo_parts = []
        idx_hi_parts = []
        scal = np.zeros((nchunks, P, G * 16), np.float32)
        for g in range(ng):
            ci, gj = g // G, g % G
            kl, kh = int(K_lo[g]), int(K_hi[g])
            slo = np.zeros((kl, P), np.int64)
            shi = np.full((kh, P), dummy_hi, np.int64)
            for p in range(P):
                j = g * P + p
                lo, hi = cc["lo"][j], cc["hi"][j]
                nl_, nh_ = len(lo), len(hi)
                if nl_ > 0:
                    slo[:nl_, p] = lo
                    slo[nl_:, p] = lo[0]
                # else stays 0 (row 0; dup + sum-corrected)
                if nh_ > 0:
                    shi[:nh_, p] = hi - half
                    shi[nh_:, p] = hi[0] - half
                cntj = cc["cnt"][cc["perm"][j]] if j < nown else 0
                safe = max(cntj, 1)
                logd = np.log(safe + 1.0)
                o = gj * 16
                scal[ci, p, o + 0] = -(kl - nl_)          # npadlo
                scal[ci, p, o + 1] = -(kh - nh_)          # npadhi
                scal[ci, p, o + 2] = 0.0 if nl_ > 0 else BIG   # mlo_big
                scal[ci, p, o + 3] = 0.0 if nh_ > 0 else BIG   # mhi_big
                scal[ci, p, o + 4] = 1.0 / safe           # recip
                scal[ci, p, o + 5] = 1.0 if cntj > 0 else 0.0  # nonempty
                scal[ci, p, o + 6] = logd / avg_log       # f1
                scal[ci, p, o + 7] = avg_log / logd       # f2
                scal[ci, p, o + 8] = float(cntj)          # cnt
            idx_lo_parts.append(_wrap16(slo.reshape(-1).astype(np.int16)))
            idx_hi_parts.append(_wrap16(shi.reshape(-1).astype(np.int16)))
        perm_pad = np.concatenate([cc["perm"], np.zeros(nownp - nown, np.int64)])
        # x2 perm gather: perm position j -> local row perm_pad[j] (pad -> nown)
        x2src = perm_pad.copy()
        x2src[nown:] = nown  # zeroed pad row
        x2pidx = _wrap16(x2src.astype(np.int16))
        # h scatter: perm position j -> natural row perm_pad[j] (pad -> dummy)
        hdest = perm_pad.copy()
        hdest[nown:] = nownp  # dummy row
        h_idx = _wrap16(hdest.astype(np.int16))
        out_cores.append(dict(
            idx_lo=np.concatenate(idx_lo_parts, axis=1),
            idx_hi=np.concatenate(idx_hi_parts, axis=1),
            scal=scal, perm_pad=perm_pad, x2pidx=x2pidx, h_idx=h_idx,
        ))
    return out_cores, K_lo, K_hi, ng, nownp


def _prep_weights(inputs, l):
    """Per-layer/direction packed weights (numpy)."""
    w = {}
    for d, tag in enumerate(("in", "out")):
        preW = np.asarray(inputs[f"{tag}_pre_W"][l], np.float32)    # [256,128]
        preB = np.asarray(inputs[f"{tag}_pre_b"][l], np.float32)    # [128]
        postW = np.asarray(inputs[f"{tag}_post_W"][l], np.float32)  # [2048,64]
        postB = np.asarray(inputs[f"{tag}_post_b"][l], np.float32)  # [64]
        linW = np.asarray(inputs[f"{tag}_lin_W"][l], np.float32)    # [64,64]
        linB = np.asarray(inputs[f"{tag}_lin_b"][l], np.float32)    # [64]
        w[(d, "wbot")] = preW[F:2 * F]                               # [128,128]
        w[(d, "acatw")] = np.concatenate([preW[0:F], postW[0:F]], axis=1)  # [128,192]
        w[(d, "acatb")] = np.tile(np.concatenate([preB, postB])[None, :], (P, 1))
        pp = np.zeros((F, 5 * 192), np.float32)
        for p_ in range(5):
            for k in range(3):
                rows = postW[F + k * 5 * F + p_ * F: F + k * 5 * F + (p_ + 1) * F]
                pp[:, p_ * 192 + k * 64: p_ * 192 + (k + 1) * 64] = rows
        w[(d, "ppw")] = pp
        w[(d, "linw")] = linW
        w[(d, "linb")] = np.tile(linB[None, :], (P, 1))
    w["combw"] = np.asarray(inputs["comb_W"][l], np.float32)         # [256,128]
    w["combb"] = np.tile(np.asarray(inputs["comb_b"][l], np.float32)[None, :], (P, 1))
    return w


# -------------------------------------------------------------- device build

def build_program(meta):
    """Build the SPMD bass program (shared by all 8 cores)."""
    n_nodes = meta["n_nodes"]
    half = meta["half"]
    tbl = meta["tbl"]
    nown = meta["nown"]
    nownp = meta["nownp"]
    ng = meta["ng"]
    K_lo = meta["K_lo"]    # dict d -> [ng]
    K_hi = meta["K_hi"]
    chunks = _chunks_of(ng)
    nchunks = len(chunks)
    sum_klo = {d: int(K_lo[d].sum()) for d in (0, 1)}
    sum_khi = {d: int(K_hi[d].sum()) for d in (0, 1)}
    maxKtot = max(int((K_lo[d] + K_hi[d]).max()) for d in (0, 1))
    maxKhalf = max(max(int(K_lo[d].max()), int(K_hi[d].max())) for d in (0, 1))
    HT = nownp + P                  # h table rows (incl dummy)

    nc = bacc.Bacc("TRN2", target_bir_lowering=False, debug=False,
                   num_devices=NCORES, num_swdge_queues=NQ)

    # ---- DRAM I/O
    xTown = nc.dram_tensor("xTown", [P, nownp], FP32, kind="ExternalInput")
    xTownP = {d: nc.dram_tensor(f"xTownP{d}", [P, nownp], FP32, kind="ExternalInput") for d in (0, 1)}
    idx_lo = {d: nc.dram_tensor(f"idx_lo{d}", [P, sum_klo[d] * 8], I16, kind="ExternalInput") for d in (0, 1)}
    idx_hi = {d: nc.dram_tensor(f"idx_hi{d}", [P, sum_khi[d] * 8], I16, kind="ExternalInput") for d in (0, 1)}
    scal_t = {d: nc.dram_tensor(f"scal{d}", [nchunks, P, G * 16], FP32, kind="ExternalInput") for d in (0, 1)}
    x2pidx_t = {d: nc.dram_tensor(f"x2pidx{d}", [P, nownp // 16], I16, kind="ExternalInput") for d in (0, 1)}
    h_idx_t = {d: nc.dram_tensor(f"hidx{d}", [P, nownp // 16], I16, kind="ExternalInput") for d in (0, 1)}
    wbot_t = nc.dram_tensor("wbot", [LAYERS, 2, F, F], FP32, kind="ExternalInput")
    acatw_t = nc.dram_tensor("acatw", [LAYERS, 2, F, 192], FP32, kind="ExternalInput")
    acatb_t = nc.dram_tensor("acatb", [LAYERS, 2, P, 192], FP32, kind="ExternalInput")
    ppw_t = nc.dram_tensor("ppw", [LAYERS, 2, F, 5 * 192], FP32, kind="ExternalInput")
    linw_t = nc.dram_tensor("linw", [LAYERS, 2, 64, 64], FP32, kind="ExternalInput")
    linb_t = nc.dram_tensor("linb", [LAYERS, 2, P, 64], FP32, kind="ExternalInput")
    combw_t = nc.dram_tensor("combw", [LAYERS, 256, F], FP32, kind="ExternalInput")
    combb_t = nc.dram_tensor("combb", [LAYERS, P, F], FP32, kind="ExternalInput")
    headw_t = nc.dram_tensor("headw", [F, 8], FP32, kind="ExternalInput")
    headb_t = nc.dram_tensor("headb", [P, 8], FP32, kind="ExternalInput")
    out_t = nc.dram_tensor("out", [nown, 8], FP32, kind="ExternalOutput")

    # ---- DRAM internal
    Bt = {d: nc.dram_tensor(f"Bt{d}", [tbl, F], FP32, addr_space="Shared") for d in (0, 1)}
    Btown = {d: nc.dram_tensor(f"Btown{d}", [nown, F], FP32) for d in (0, 1)}
    h_tab = {d: nc.dram_tensor(f"htab{d}", [HT, 64], FP32) for d in (0, 1)}
    x2own = nc.dram_tensor("x2own", [nownp, F], FP32)

    AF = mybir.ActivationFunctionType
    OP = mybir.AluOpType
    AX = mybir.AxisListType

    with TileContext(nc) as tc:
        with tc.tile_pool(name="const", bufs=1) as constp, \
             tc.tile_pool(name="wts", bufs=1) as wtsp, \
             tc.tile_pool(name="xt", bufs=2) as xtp, \
             tc.tile_pool(name="ip", bufs=3) as ipool, \
             tc.tile_pool(name="gath", bufs=2) as gathp, \
             tc.tile_pool(name="sqp", bufs=2) as sqp, \
             tc.tile_pool(name="chk", bufs=1) as chkp, \
             tc.tile_pool(name="s0p", bufs=2) as s0p, \
             tc.tile_pool(name="axp", bufs=2) as axp, \
             tc.tile_pool(name="pys", bufs=2) as pysp, \
             tc.tile_pool(name="nl", bufs=2) as nlp, \
             tc.tile_pool(name="ps", bufs=2, space="PSUM") as psp, \
             tc.tile_pool(name="psa", bufs=2, space="PSUM") as psap:

            ident = constp.tile([P, P], FP32)
            make_identity(nc, ident[:])
            zeros = constp.tile([P, 512], FP32)
            nc.vector.memset(zeros[:], 0.0)
            eps_col = constp.tile([P, 1], FP32)
            nc.vector.memset(eps_col[:], EPS)
            ones1 = constp.tile([1, P], FP32)
            nc.vector.memset(ones1[:], 1.0)

            def load_w(pool, dram_ap, shape, tag):
                t = pool.tile(shape, FP32, tag=tag, name=tag)
                nc.sync.dma_start(out=t[:], in_=dram_ap)
                return t

            headw_sb = load_w(constp, headw_t[:], [F, 8], "headw")
            headb_sb = load_w(constp, headb_t[:], [P, 8], "headb")

            def zero_dram_rows(tensor, r0, r1, width):
                if r1 <= r0:
                    return
                flat = tensor[r0:r1, :].rearrange("n f -> (n f)")
                total = (r1 - r0) * width
                assert total % P == 0
                per = total // P
                vv = flat.rearrange("(p x) -> p x", p=P)
                off = 0
                while off < per:
                    w = min(512, per - off)
                    nc.sync.dma_start(out=vv[:, off:off + w], in_=zeros[:, 0:w])
                    off += w

            def transpose_to_sbuf(src_ap, tag):
                """PE transpose src [128, <=128] -> sbuf (scalar copy out)."""
                pt = psp.tile([P, P], FP32, tag="ptr")
                nc.tensor.transpose(out=pt[:], in_=src_ap, identity=ident[:])
                sb = xtp.tile([P, P], FP32, tag=tag, name=tag)
                nc.scalar.copy(sb[:], pt[:])
                return sb

            # one-time zeroing
            for d in (0, 1):
                zero_dram_rows(Bt[d], n_nodes, tbl, F)
            zero_dram_rows(x2own, nown, nownp, F)

            for l in range(LAYERS):
                wbot_sb = {d: load_w(wtsp, wbot_t[l, d], [F, F], f"wbot{d}") for d in (0, 1)}
                combw1_sb = load_w(wtsp, combw_t[l, 0:F, :], [F, F], "combw1")
                combw2_sb = load_w(wtsp, combw_t[l, F:256, :], [F, F], "combw2")
                combb_sb = load_w(wtsp, combb_t[l], [P, F], "combb")

                for d in (0, 1):
                    zero_dram_rows(h_tab[d], 0, HT, 64)

                # ---------- phase A: sharded B-table build + AllGather
                for t in range(ng):
                    if l == 0:
                        lhsT = xtp.tile([P, P], FP32, tag="xtt", name="xtt")
                        nc.sync.dma_start(out=lhsT[:], in_=xTown[:, t * P:(t + 1) * P])
                    else:
                        x2t = xtp.tile([P, P], FP32, tag="x2t", name="x2t")
                        nc.sync.dma_start(out=x2t[:], in_=x2own[t * P:(t + 1) * P, :])
                        lhsT = transpose_to_sbuf(x2t[:], "xtt")
                    rows_o = min(P, nown - t * P)
                    for d in (0, 1):
                        pb = psp.tile([P, P], FP32, tag="pb")
                        nc.tensor.matmul(out=pb[:], lhsT=lhsT[:], rhs=wbot_sb[d][:],
                                         start=True, stop=True)
                        sb = xtp.tile([P, P], FP32, tag="bsb", name="bsb")
                        nc.scalar.copy(sb[:], pb[:])
                        nc.sync.dma_start(out=Btown[d][t * P:t * P + rows_o, :],
                                          in_=sb[:rows_o, :])
                for d in (0, 1):
                    nc.gpsimd.collective_compute(
                        "AllGather", OP.bypass,
                        replica_groups=[list(range(NCORES))],
                        ins=[Btown[d][:]], outs=[Bt[d][0:n_nodes, :]])

                for d in (0, 1):
                    acatw_sb = load_w(wtsp, acatw_t[l, d], [F, 192], "acatw")
                    acatb_sb = load_w(wtsp, acatb_t[l, d], [P, 192], "acatb")
                    ppw_sb = load_w(wtsp, ppw_t[l, d], [F, 5 * 192], "ppw")
                    linw_sb = load_w(wtsp, linw_t[l, d], [64, 64], "linw")
                    linb_sb = load_w(wtsp, linb_t[l, d], [P, 64], "linb")
                    hix = wtsp.tile([P, nownp // 16], I16, tag="hix", name="hix")
                    nc.sync.dma_start(out=hix[:], in_=h_idx_t[d][:])
                    if l == 1:
                        x2pix = wtsp.tile([P, nownp // 16], I16, tag="x2pix", name="x2pix")
                        nc.sync.dma_start(out=x2pix[:], in_=x2pidx_t[d][:])

                    off_lo = 0
                    off_hi = 0
                    kl_arr, kh_arr = K_lo[d], K_hi[d]
                    hsb_prev = None  # deferred chunk scatter
                    v = nc.vector

                    def scatter_chunk(entry):
                        hs, cci, gcc = entry
                        nc.gpsimd.dma_scatter_add(
                            out_ap=h_tab[d][:],
                            in_ap=hs[:, 0:gcc * 64].rearrange("p (c w) -> p c w", w=64),
                            idxs_ap=hix[:, cci * G * 8:cci * G * 8 + gcc * 8],
                            num_idxs=gcc * P, num_idxs_reg=gcc * P,
                            elem_size=64, single_packet=False, queue_num=0)

                    for ci in range(nchunks):
                        Gc = chunks[ci]
                        g0 = ci * G

                        if l == 1:
                            x2g = ipool.tile([P, Gc * F], FP32, tag="x2g",
                                             padded_shape=[P, G * F], bufs=2, name="x2g")
                            nc.gpsimd.dma_gather(
                                out_ap=x2g[:].rearrange("p (c w) -> p c w", w=F),
                                in_ap=x2own[:], idxs_ap=x2pix[:, ci * G * 8: ci * G * 8 + Gc * 8],
                                num_idxs=Gc * P, num_idxs_reg=Gc * P,
                                elem_size=F, single_packet=False, queue_num=0)

                        scs = ipool.tile([P, G * 16], FP32, tag="scs", name="scs")
                        nc.sync.dma_start(out=scs[:], in_=scal_t[d][ci])

                        st = chkp.tile([P, 7 * G * F], FP32, tag="st", bufs=2, name="st")
                        s0c = s0p.tile([P, 2 * G * F], FP32, tag="s0c", name="s0c")

                        # ---- per-group: gathers + stash + squares + reduces
                        for gj in range(Gc):
                            g = g0 + gj
                            KL, KH = int(kl_arr[g]), int(kh_arr[g])
                            Wd = (KL + KH) * F
                            il = ipool.tile([P, KL * 8], I16, tag="il",
                                            padded_shape=[P, maxKhalf * 8], name="il")
                            nc.sync.dma_start(out=il[:], in_=idx_lo[d][:, off_lo:off_lo + KL * 8])
                            ih = ipool.tile([P, KH * 8], I16, tag="ih",
                                            padded_shape=[P, maxKhalf * 8], name="ih")
                            nc.sync.dma_start(out=ih[:], in_=idx_hi[d][:, off_hi:off_hi + KH * 8])
                            off_lo += KL * 8
                            off_hi += KH * 8
                            gt = gathp.tile([P, Wd], FP32, tag="gt",
                                            padded_shape=[P, maxKtot * F], name="gt")
                            nc.gpsimd.dma_gather(
                                out_ap=gt[:, 0:KL * F].rearrange("p (k f) -> p k f", f=F),
                                in_ap=Bt[d][0:half, :], idxs_ap=il[:],
                                num_idxs=KL * P, num_idxs_reg=KL * P,
                                elem_size=F, single_packet=False, queue_num=0)
                            nc.gpsimd.dma_gather(
                                out_ap=gt[:, KL * F:Wd].rearrange("p (k f) -> p k f", f=F),
                                in_ap=Bt[d][half:tbl, :], idxs_ap=ih[:],
                                num_idxs=KH * P, num_idxs_reg=KH * P,
                                elem_size=F, single_packet=False, queue_num=0)
                            # stash slot-0 rows (h-major layout: [2, G, F])
                            nc.scalar.copy(s0c[:, gj * F:(gj + 1) * F], gt[:, 0:F])
                            nc.scalar.copy(s0c[:, (G + gj) * F:(G + gj + 1) * F],
                                           gt[:, KL * F:KL * F + F])
                            # combined sum over lo+hi
                            v.tensor_reduce(st[:, (0 * G + gj) * F:(0 * G + gj + 1) * F],
                                            gt[:, 0:Wd].rearrange("p (k f) -> p f k", f=F),
                                            axis=AX.X, op=OP.add)
                            sqlo = sqp.tile([P, KL * F], FP32, tag="sq",
                                            padded_shape=[P, maxKhalf * F], name="sqlo")
                            nc.scalar.activation(sqlo[:], gt[:, 0:KL * F], AF.Square)
                            v.tensor_reduce(st[:, (1 * G + gj) * F:(1 * G + gj + 1) * F],
                                            sqlo[:].rearrange("p (k f) -> p f k", f=F),
                                            axis=AX.X, op=OP.add)
                            sqhi = sqp.tile([P, KH * F], FP32, tag="sq",
                                            padded_shape=[P, maxKhalf * F], name="sqhi")
                            nc.scalar.activation(sqhi[:], gt[:, KL * F:Wd], AF.Square)
                            v.tensor_reduce(st[:, (2 * G + gj) * F:(2 * G + gj + 1) * F],
                                            sqhi[:].rearrange("p (k f) -> p f k", f=F),
                                            axis=AX.X, op=OP.add)
                            lo_v = gt[:, 0:KL * F].rearrange("p (k f) -> p f k", f=F)
                            hi_v = gt[:, KL * F:Wd].rearrange("p (k f) -> p f k", f=F)
                            v.tensor_reduce(st[:, (3 * G + gj) * F:(3 * G + gj + 1) * F],
                                            lo_v, axis=AX.X, op=OP.min)
                            v.tensor_reduce(st[:, (4 * G + gj) * F:(4 * G + gj + 1) * F],
                                            hi_v, axis=AX.X, op=OP.min)
                            v.tensor_reduce(st[:, (5 * G + gj) * F:(5 * G + gj + 1) * F],
                                            lo_v, axis=AX.X, op=OP.max)
                            v.tensor_reduce(st[:, (6 * G + gj) * F:(6 * G + gj + 1) * F],
                                            hi_v, axis=AX.X, op=OP.max)

                        # ---- A|X0 for the chunk (perm order), bias via PE
                        AXc = axp.tile([P, G * 192], FP32, tag="AXc", name="AXc")
                        for gj in range(Gc):
                            g = g0 + gj
                            if l == 0:
                                lhsT = xtp.tile([P, P], FP32, tag="xtt", name="xtt")
                                nc.sync.dma_start(out=lhsT[:],
                                                  in_=xTownP[d][:, g * P:(g + 1) * P])
                            else:
                                lhsT = transpose_to_sbuf(x2g[:, gj * F:(gj + 1) * F], "xtt")
                            pa = psap.tile([P, 192], FP32, tag="pa")
                            nc.tensor.matmul(out=pa[:], lhsT=lhsT[:], rhs=acatw_sb[:],
                                             start=True, stop=False)
                            nc.tensor.matmul(out=pa[:], lhsT=ones1[:], rhs=acatb_sb[0:1, :],
                                             start=False, stop=True)
                            nc.scalar.copy(AXc[:, gj * 192:(gj + 1) * 192], pa[:])

                        # ---- batched tail over the chunk
                        q0 = chkp.tile([P, 2 * G * F], FP32, tag="q0", name="q0")
                        nc.scalar.activation(q0[:, 0:Gc * F], s0c[:, 0:Gc * F], AF.Square)
                        nc.scalar.activation(q0[:, G * F:(G + Gc) * F],
                                             s0c[:, G * F:(G + Gc) * F], AF.Square)

                        scs_r = scs[:].rearrange("p (g s) -> p g s", s=16)

                        def bc(k, w=F):
                            return scs_r[:, 0:Gc, k:k + 1].broadcast_to((P, Gc, w))

                        def r3(tile_ap, w=F):
                            return tile_ap.rearrange("p (g f) -> p g f", f=w)

                        s0lo_v = r3(s0c[:, 0:Gc * F])
                        s0hi_v = r3(s0c[:, G * F:(G + Gc) * F])
                        q0lo_v = r3(q0[:, 0:Gc * F])
                        q0hi_v = r3(q0[:, G * F:(G + Gc) * F])

                        def scr(tag):
                            t = chkp.tile([P, G * F], FP32, tag=tag, name=tag)
                            return t, r3(t[:, 0:Gc * F])

                        t1, t1r = scr("t1")
                        t2, t2r = scr("t2")
                        t3, t3r = scr("t3")
                        t4, t4r = scr("t4")
                        Sb, Sbr = scr("Sb")
                        SQb, SQbr = scr("SQb")
                        MN, MNr = scr("MN")
                        MX, MXr = scr("MX")
                        so = chkp.tile([P, 5 * G * F], FP32, tag="so", bufs=2, name="so")

                        def sov(si):
                            return r3(so[:, si * G * F:(si * G + Gc) * F])

                        mean_o, sfull_o, mn_o, mx_o = sov(0), sov(1), sov(3), sov(4)

                        def stv(s):
                            return r3(st[:, s * G * F:(s * G + Gc) * F])

                        AXr = r3(AXc[:, 0:Gc * 192], 192)
                        A_v = AXr[:, :, 0:F]
                        X0_v = AXr[:, :, F:192]

                        v.tensor_tensor(t1r, s0lo_v, bc(0), op=OP.mult)
                        v.tensor_tensor(t2r, s0hi_v, bc(1), op=OP.mult)
                        v.tensor_tensor(t3r, t1r, t2r, op=OP.add)
                        v.tensor_tensor(Sbr, stv(0), t3r, op=OP.add)
                        v.tensor_tensor(t1r, q0lo_v, bc(0), op=OP.mult)
                        v.tensor_tensor(t2r, q0hi_v, bc(1), op=OP.mult)
                        v.tensor_tensor(t3r, t1r, t2r, op=OP.add)
                        v.tensor_tensor(t4r, stv(1), stv(2), op=OP.add)
                        v.tensor_tensor(SQbr, t4r, t3r, op=OP.add)
                        v.tensor_tensor(t1r, stv(3), bc(2), op=OP.add)
                        v.tensor_tensor(t2r, stv(4), bc(3), op=OP.add)
                        v.tensor_tensor(MNr, t1r, t2r, op=OP.min)
                        v.tensor_tensor(t1r, stv(5), bc(2), op=OP.subtract)
                        v.tensor_tensor(t2r, stv(6), bc(3), op=OP.subtract)
                        v.tensor_tensor(MXr, t1r, t2r, op=OP.max)
                        v.tensor_tensor(t1r, A_v, bc(8), op=OP.mult)
                        v.tensor_tensor(sfull_o, t1r, Sbr, op=OP.add)
                        v.tensor_tensor(mean_o, sfull_o, bc(4), op=OP.mult)
                        v.tensor_tensor(t2r, Sbr, bc(4), op=OP.mult)
                        v.tensor_tensor(t3r, SQbr, bc(4), op=OP.mult)
                        v.tensor_tensor(t4r, t2r, t2r, op=OP.mult)
                        v.tensor_tensor(t1r, t3r, t4r, op=OP.subtract)
                        v.tensor_scalar_max(t2[:, 0:Gc * F], t1[:, 0:Gc * F], 0.0)
                        nc.scalar.activation(so[:, 2 * G * F:(2 * G + Gc) * F],
                                             t2[:, 0:Gc * F], AF.Sqrt,
                                             bias=eps_col[:, 0:1])
                        v.tensor_tensor(t3r, A_v, MNr, op=OP.add)
                        v.tensor_tensor(mn_o, t3r, bc(5), op=OP.mult)
                        v.tensor_tensor(t4r, A_v, MXr, op=OP.add)
                        v.tensor_tensor(mx_o, t4r, bc(5), op=OP.mult)

                        # ---- per-group post matmuls
                        pysc = pysp.tile([P, G * 192], FP32, tag="pysc", name="pysc")
                        for gj in range(Gc):
                            py = psap.tile([P, 192], FP32, tag="pa")
                            for pi in range(5):
                                src = so[:, (pi * G + gj) * F:(pi * G + gj + 1) * F]
                                pt = psp.tile([P, P], FP32, tag="ptr")
                                nc.tensor.transpose(out=pt[:], in_=src, identity=ident[:])
                                partT = xtp.tile([P, P], FP32, tag="partT", name="partT")
                                v.tensor_copy(partT[:], pt[:])
                                nc.tensor.matmul(out=py[:], lhsT=partT[:],
                                                 rhs=ppw_sb[:, pi * 192:(pi + 1) * 192],
                                                 start=(pi == 0), stop=(pi == 4))
                            nc.scalar.copy(pysc[:, gj * 192:(gj + 1) * 192], py[:])
                        pys_r = r3(pysc[:, 0:Gc * 192], 192)
                        yt1r = r3(t1[:, 0:Gc * 64], 64)
                        yt2r = r3(t2[:, 0:Gc * 64], 64)
                        yt3r = r3(t3[:, 0:Gc * 64], 64)
                        y64 = pysp.tile([P, G * 64], FP32, tag="y64", name="y64")
                        y64r = r3(y64[:, 0:Gc * 64], 64)
                        v.tensor_tensor(yt1r, pys_r[:, :, 64:128], bc(6, 64), op=OP.mult)
                        v.tensor_tensor(yt2r, yt1r, pys_r[:, :, 0:64], op=OP.add)
                        v.tensor_tensor(yt1r, pys_r[:, :, 128:192], bc(7, 64), op=OP.mult)
                        v.tensor_tensor(yt3r, yt1r, X0_v, op=OP.add)
                        v.tensor_tensor(y64r, yt2r, yt3r, op=OP.add)

                        # ---- per-group lin (+bias via PE) + relu straight to hsbc
                        hsbc = pysp.tile([P, G * 64], FP32, tag="hsbc", name="hsbc")
                        for gj in range(Gc):
                            pt = psp.tile([P, P], FP32, tag="ptr")
                            nc.tensor.transpose(out=pt[:64, :], in_=y64[:, gj * 64:(gj + 1) * 64],
                                                identity=ident[:])
                            ylhs = xtp.tile([64, P], FP32, tag="ylhs", name="ylhs")
                            v.tensor_copy(ylhs[:], pt[:64, :])
                            pz = psp.tile([P, 64], FP32, tag="pz")
                            nc.tensor.matmul(out=pz[:], lhsT=ylhs[:], rhs=linw_sb[:],
                                             start=True, stop=False)
                            nc.tensor.matmul(out=pz[:], lhsT=ones1[:], rhs=linb_sb[0:1, 0:64],
                                             start=False, stop=True)
                            nc.scalar.activation(hsbc[:, gj * 64:(gj + 1) * 64], pz[:], AF.Relu)

                        # ---- scatter previous chunk, defer this one
                        if hsb_prev is not None:
                            scatter_chunk(hsb_prev)
                        hsb_prev = (hsbc, ci, Gc)

                    if hsb_prev is not None:
                        scatter_chunk(hsb_prev)
                        hsb_prev = None

                # ---------- phase D: comb (+ head when l==1)
                for g in range(ng):
                    rows = min(P, nown - g * P)
                    if l == 0:
                        xlhs = xtp.tile([P, P], FP32, tag="xtt", name="xtt")
                        nc.sync.dma_start(out=xlhs[:],
                                          in_=xTown[:, g * P:(g + 1) * P])
                    else:
                        x2t = xtp.tile([P, P], FP32, tag="x2t", name="x2t")
                        nc.sync.dma_start(out=x2t[:], in_=x2own[g * P:(g + 1) * P, :])
                        xlhs = transpose_to_sbuf(x2t[:], "xtt")
                    hcatT = xtp.tile([P, P], FP32, tag="hcatT", name="hcatT")
                    for d in (0, 1):
                        htile = xtp.tile([P, 64], FP32, tag="htile", name="htile")
                        nc.sync.dma_start(out=htile[:], in_=h_tab[d][g * P:(g + 1) * P, :])
                        pt = psp.tile([P, P], FP32, tag="ptr")
                        nc.tensor.transpose(out=pt[:64, :], in_=htile[:], identity=ident[:])
                        nc.scalar.copy(hcatT[d * 64:(d + 1) * 64, :], pt[:64, :])
                    pc = psp.tile([P, P], FP32, tag="pb")
                    nc.tensor.matmul(out=pc[:], lhsT=xlhs[:], rhs=combw1_sb[:],
                                     start=True, stop=False)
                    nc.tensor.matmul(out=pc[:], lhsT=hcatT[:], rhs=combw2_sb[:],
                                     start=False, stop=False)
                    nc.tensor.matmul(out=pc[:], lhsT=ones1[:], rhs=combb_sb[0:1, :],
                                     start=False, stop=True)
                    if l == 0:
                        x2n = nlp.tile([P, F], FP32, tag="x2n", name="x2n")
                        nc.scalar.activation(x2n[:], pc[:], AF.Relu)
                        nc.sync.dma_start(out=x2own[g * P:g * P + rows, :],
                                          in_=x2n[:rows, :])
                    else:
                        x3 = nlp.tile([P, F], FP32, tag="x3", name="x3")
                        nc.scalar.activation(x3[:], pc[:], AF.Relu)
                        pt = psp.tile([P, P], FP32, tag="ptr")
                        nc.tensor.transpose(out=pt[:], in_=x3[:], identity=ident[:])
                        x3T = xtp.tile([P, P], FP32, tag="x3T", name="x3T")
                        nc.scalar.copy(x3T[:], pt[:])
                        ph = psp.tile([P, 8], FP32, tag="pz")
                        nc.tensor.matmul(out=ph[:], lhsT=x3T[:], rhs=headw_sb[:],
                                         start=True, stop=False)
                        nc.tensor.matmul(out=ph[:], lhsT=ones1[:], rhs=headb_sb[0:1, :],
                                         start=False, stop=True)
                        ot = nlp.tile([P, 8], FP32, tag="ot", name="ot")
                        nc.scalar.copy(ot[:], ph[:])
                        nc.sync.dma_start(out=out_t[g * P:g * P + rows, :],
                                          in_=ot[:rows, :])

    # Spread SWDGE DMAs across the 4 queues, consistently with the DMASW
    # semaphore lane each instruction was assigned (different queues must
    # not increment the same sem — shadow sem tracking).
    from concourse.tile_scheduler import PROC_NAME_TO_IDX
    sw0 = PROC_NAME_TO_IDX["DMASW0"]
    npatched = 0
    for fn in nc.m.functions:
        for bb in fn.blocks:
            for ins in bb.instructions:
                proc = ins.bass_scheduled_proc
                if proc is not None and sw0 <= proc < sw0 + 8 and hasattr(ins, "queue_num"):
                    ins.queue_num = (proc - sw0) % NQ
                    npatched += 1
    assert npatched > 0, "no SWDGE DMAs patched - scheduling pass not run?"

    nc.finalize()
    return nc


# ----------------------------------------------------------------- kernel()

def _install_ntff_hook():
    """Register the axon NTFF profile hook if the image's antenv lacks it."""
    import sys
    import types
    try:
        from antenv.axon_hooks import get_axon_ntff_profile_hook  # noqa: F401
        return
    except ImportError:
        pass
    try:
        mod = types.ModuleType("antenv.axon_hooks")
        hook = {"h": None}
        mod.set_axon_ntff_profile_hook = lambda h: hook.__setitem__("h", h)
        mod.get_axon_ntff_profile_hook = lambda: hook["h"]
        sys.modules["antenv.axon_hooks"] = mod
        import antenv
        antenv.axon_hooks = mod
        from trn_agent_boot.trn_boot import _ntff_profile_via_ctypes
        mod.set_axon_ntff_profile_hook(
            _ntff_profile_via_ctypes("/opt/axon/libaxon_pjrt.so"))
    except Exception:
        pass


def _prep_all(inputs):
    x = np.asarray(inputs["x"], np.float32)
    n_nodes, f = x.shape
    assert f == F
    assert n_nodes % NCORES == 0
    nown = n_nodes // NCORES
    half = ((n_nodes // 2 + 1 + P - 1) // P) * P
    assert half < 32768 and 2 * half > n_nodes
    tbl = 2 * half

    avg_in = float(np.asarray(inputs["avg_in"]))
    avg_out = float(np.asarray(inputs["avg_out"]))

    prep = {}
    Ksched = {}
    for d, (ei, avg) in enumerate(
            ((inputs["edge_index_in"], avg_in), (inputs["edge_index_out"], avg_out))):
        cores, K_lo, K_hi, ng, nownp = _prep_direction(ei, n_nodes, nown, half, avg)
        prep[d] = cores
        Ksched[d] = (K_lo, K_hi)

    meta = dict(n_nodes=n_nodes, half=half, tbl=tbl, nown=nown, nownp=nownp,
                ng=ng, K_lo={d: Ksched[d][0] for d in (0, 1)},
                K_hi={d: Ksched[d][1] for d in (0, 1)})

    wl = [_prep_weights(inputs, l) for l in range(LAYERS)]
    shared = dict(
        wbot=np.stack([np.stack([wl[l][(d, "wbot")] for d in (0, 1)]) for l in range(LAYERS)]),
        acatw=np.stack([np.stack([wl[l][(d, "acatw")] for d in (0, 1)]) for l in range(LAYERS)]),
        acatb=np.stack([np.stack([wl[l][(d, "acatb")] for d in (0, 1)]) for l in range(LAYERS)]),
        ppw=np.stack([np.stack([wl[l][(d, "ppw")] for d in (0, 1)]) for l in range(LAYERS)]),
        linw=np.stack([np.stack([wl[l][(d, "linw")] for d in (0, 1)]) for l in range(LAYERS)]),
        linb=np.stack([np.stack([wl[l][(d, "linb")] for d in (0, 1)]) for l in range(LAYERS)]),
        combw=np.stack([wl[l]["combw"] for l in range(LAYERS)]),
        combb=np.stack([wl[l]["combb"] for l in range(LAYERS)]),
        headw=np.asarray(inputs["head_W"], np.float32),
        headb=np.tile(np.asarray(inputs["head_b"], np.float32)[None, :], (P, 1)),
    )

    in_maps = []
    for c in range(NCORES):
        nownp = meta["nownp"]
        xTown_np = np.zeros((P, nownp), np.float32)
        xown = x[c * nown:(c + 1) * nown]
        xTown_np[:, :nown] = xown.T
        m = dict(xTown=xTown_np, **shared)
        for d in (0, 1):
            pc = prep[d][c]
            pp = pc["perm_pad"]
            xP = np.zeros((P, nownp), np.float32)
            valid = pp < nown
            cols = np.where(valid[:nown])[0]
            xP[:, :nown] = xown[pp[:nown]].T
            m[f"xTownP{d}"] = xP
            m[f"idx_lo{d}"] = pc["idx_lo"]
            m[f"idx_hi{d}"] = pc["idx_hi"]
            m[f"scal{d}"] = pc["scal"]
            m[f"x2pidx{d}"] = pc["x2pidx"]
            m[f"hidx{d}"] = pc["h_idx"]
        in_maps.append(m)
    return meta, in_maps


def kernel(**inputs):
    meta, in_maps = _prep_all(inputs)
    nc = build_program(meta)
    trace = bool(int(os.environ.get("PNA_TRACE", "0")))
    if trace:
        _install_ntff_hook()
    res = run_bass_kernel_spmd(nc, in_maps, core_ids=list(range(NCORES)),
                               trace=trace)
    if trace and res.exec_time_ns is not None:
        print(f"HW exec time: {res.exec_time_ns} ns")
    out = np.concatenate([res.results[c]["out"] for c in range(NCORES)], axis=0)
    return out.astype(np.float32)


def kernel_sim(**inputs):
    """CoreSim path for debugging (small inputs)."""
    from concourse.bass_interp import MultiCoreSim
    meta, in_maps = _prep_all(inputs)
    nc = build_program(meta)
    sim = MultiCoreSim(nc, num_cores=NCORES, trace=False,
                       require_finite=False, require_nnan=False)
    for c in range(NCORES):
        cs = sim.cores[c]
        for nm, val in in_maps[c].items():
            cs.tensor(nm)[:] = val
    sim.simulate(check_with_hw=False)
    nown = meta["nown"]
    out = np.concatenate([np.array(sim.cores[c].tensor("out")) for c in range(NCORES)], axis=0)
    return out.astype(np.float32)
